# revision 28
# baseline (speedup 1.0000x reference)
"""Trainium2 Bass kernel for causal multi-head attention (no KV cache).

Problem: x[1,4096,1024], w_qkv[3072,1024], w_out[1024,1024], 16 heads, hd=64.
Sharding: tensor-parallel over heads — each of the 8 cores owns 2 heads.
Per core: QKV projection for its heads, causal softmax(QK^T/8)V, and the
partial output projection  y_c @ w_out[:, c*128:(c+1)*128].T  -> [4096,1024].
Host sums the 8 fp32 partials.

Device layout choices:
  - x is shipped pre-transposed (xT [1024,4096], bf16): the QKV matmuls
    contract over the model dim, which must live on SBUF partitions.
  - q^T and k^T are produced directly as [64, T] (head dim on partitions) by
    making the projection-weight slice the stationary operand.  1/sqrt(hd) is
    folded into Wq on the host.
  - scores are computed transposed, S^T[k,q], so the exp output is already the
    lhsT the PV matmul needs — no on-device transposes in the inner loop.
  - softmax denominator comes free from a ones-column appended to V.
  - causal masking: k-tiles strictly above the diagonal are skipped, the
    diagonal tiles are free-dim-trimmed and masked with a static [128,128]
    triangular multiply.
"""

import os
import sys

import numpy as np

for _p in ("/opt/trn_rl_repo", "/root/.axon_site/_ro/trn_rl_repo"):
    if os.path.isdir(_p) and _p not in sys.path:
        sys.path.insert(0, _p)

import ml_dtypes

import concourse.bass as bass
import concourse.mybir as mybir
import concourse.tile as tile
from concourse import bacc, masks
from concourse._compat import with_exitstack  # noqa: F401  (parity with repo kernels)

BF16 = mybir.dt.bfloat16
F32 = mybir.dt.float32
NPBF16 = ml_dtypes.bfloat16

D_MODEL = 1024
N_HEADS = 16
HEAD_DIM = 64
N_CORES = 8
HEADS_PER_CORE = N_HEADS // N_CORES  # 2
T_FULL = 4096


def build_program(T: int = T_FULL) -> bass.Bass:
    """One NeuronCore's program; all 8 cores run it on different data."""
    assert T % 1024 == 0
    TT = T // 128          # 128-row T-tiles
    CH = D_MODEL // 128    # c-chunks of the contraction dim
    QC = T // 1024         # 1024-wide q chunks
    H2 = HEADS_PER_CORE

    nc = bacc.Bacc("TRN2", target_bir_lowering=False, debug=False)

    xt = nc.dram_tensor("xt", [D_MODEL, T], BF16, kind="ExternalInput").ap()
    # packed per c-chunk: 256 cols = [q_h0 | q_h1 | k_h0 | k_h1] (64 each)
    wqk = nc.dram_tensor("wqk", [128, CH * 256], BF16, kind="ExternalInput").ap()
    # packed per c-chunk: 128 cols = [v_h0 | v_h1]
    wv = nc.dram_tensor("wv", [128, CH * 128], BF16, kind="ExternalInput").ap()
    # w_out[:, c*128:(c+1)*128].T
    wo = nc.dram_tensor("wo", [128, D_MODEL], BF16, kind="ExternalInput").ap()
    out = nc.dram_tensor("out_partial", [T, D_MODEL], F32, kind="ExternalOutput").ap()

    with tile.TileContext(nc) as tc:
        import contextlib

        with contextlib.ExitStack() as ctx:
            const_pool = ctx.enter_context(tc.tile_pool(name="const", bufs=1))
            xt_pool = ctx.enter_context(tc.tile_pool(name="xt", bufs=1))
            qk_pool = ctx.enter_context(tc.tile_pool(name="qk", bufs=1))
            v_pool = ctx.enter_context(tc.tile_pool(name="v", bufs=1))
            y_pool = ctx.enter_context(tc.tile_pool(name="y", bufs=1))
            pt_pool = ctx.enter_context(tc.tile_pool(name="ptp", bufs=6))
            yt_pool = ctx.enter_context(tc.tile_pool(name="ytp", bufs=3))
            ob_pool = ctx.enter_context(tc.tile_pool(name="obp", bufs=3))
            rec_pool = ctx.enter_context(tc.tile_pool(name="recp", bufs=8))
            # [128,1024] fp32 = 2 banks x 2 bufs = 4 banks
            psum_mm = ctx.enter_context(tc.tile_pool(name="psmm", bufs=2, space="PSUM"))
            # [128,<=260] fp32 = 1 bank x 4 bufs = 4 banks (shared with V/transpose)
            psum_av = ctx.enter_context(tc.tile_pool(name="psav", bufs=4, space="PSUM"))

            # --- constants ---
            identity = const_pool.tile([128, 128], F32, name="identity")
            masks.make_identity(nc, identity)
            # trimask[p, g] = 1.0 where p <= g else 0  (keep k <= q)
            trimask = const_pool.tile([128, 128], BF16, name="trimask")
            masks.make_upper_triangular(nc, trimask, val=1.0, diag=True)

            # --- x^T + weight loads.  The first attention chunk only needs
            # --- x columns [0,1024), so DMA T-slice by T-slice: attention
            # --- starts ~6us in instead of waiting for the full 8MB. ---
            xt_sb = []
            for cc in range(CH):
                t = xt_pool.tile([128, T], BF16, name=f"xt{cc}", tag=f"xt{cc}")
                xt_sb.append(t)
            for cc in range(CH):
                nc.sync.dma_start(
                    xt_sb[cc][:, 0:1024], xt[cc * 128:(cc + 1) * 128, 0:1024]
                )
            wqk_sb = const_pool.tile([128, CH * 256], BF16, name="wqk_sb")
            nc.sync.dma_start(wqk_sb, wqk)
            wv_sb = const_pool.tile([128, CH * 128], BF16, name="wv_sb")
            nc.sync.dma_start(wv_sb, wv)
            for tch in range(1, T // 1024):
                for cc in range(CH):
                    nc.sync.dma_start(
                        xt_sb[cc][:, tch * 1024:(tch + 1) * 1024],
                        xt[cc * 128:(cc + 1) * 128, tch * 1024:(tch + 1) * 1024],
                    )
            wo_sb = const_pool.tile([128, D_MODEL], BF16, name="wo_sb")
            nc.sync.dma_start(wo_sb, wo)

            # --- persistent tiles ---
            q_sb = qk_pool.tile([128, T], BF16, name="q_sb", tag="q_sb")
            k_sb = qk_pool.tile([128, T], BF16, name="k_sb", tag="k_sb")
            v_sb = [[None] * TT for _ in range(H2)]
            y_sb = []
            for qt in range(TT):
                t = y_pool.tile([128, 128], F32, name=f"y{qt}", tag=f"y{qt}")
                y_sb.append(t)

            # --- emission helpers (deferred so phases interleave on PE) ---
            def emit_qkv_group(g, tch, half):
                """q^T (g=0) or k^T (g=1) for cols [1024*tch+512*half, +512).
                Uses the 1-bank psum pool so the scores double-buffer in
                psum_mm is never starved by background projection work."""
                dst = q_sb if g == 0 else k_sb
                c0 = tch * 1024 + half * 512
                ps = psum_av.tile([128, 512], F32, name=f"qkps{g}_{tch}_{half}", tag="av")
                for cc in range(CH):
                    nc.tensor.matmul(
                        ps,
                        lhsT=wqk_sb[:, cc * 256 + g * 128: cc * 256 + g * 128 + 128],
                        rhs=xt_sb[cc][:, c0:c0 + 512],
                        start=(cc == 0),
                        stop=(cc == CH - 1),
                    )
                nc.vector.tensor_copy(dst[:, c0:c0 + 512], ps)

            v_pending = {}  # kt -> deferred emit, forced before first use

            def ensure_v(kt):
                fn = v_pending.pop(kt, None)
                if fn is not None:
                    fn()

            def emit_v_tile(kt):
                """v[k-tile, 64+ones] for both heads."""
                vps = psum_av.tile([128, 128], F32, name=f"vps{kt}", tag="av")
                for cc in range(CH):
                    nc.tensor.matmul(
                        vps,
                        lhsT=xt_sb[cc][:, kt * 128:(kt + 1) * 128],
                        rhs=wv_sb[:, cc * 128:(cc + 1) * 128],
                        start=(cc == 0),
                        stop=(cc == CH - 1),
                    )
                for h in range(H2):
                    vt = v_pool.tile([128, 65], BF16, name=f"v{h}_{kt}", tag=f"v{h}_{kt}")
                    nc.vector.tensor_copy(vt[:, 0:64], vps[:, h * 64:(h + 1) * 64])
                    nc.vector.memset(vt[:, 64:65], 1.0)
                    v_sb[h][kt] = vt

            def emit_outproj(qt):
                """partial[q-tile, :] = y^T via PE transpose, then 2 matmuls."""
                tp = psum_av.tile([128, 128], F32, name=f"tp{qt}", tag="av")
                nc.tensor.transpose(tp, y_sb[qt], identity)
                yt = yt_pool.tile([128, 128], BF16, name=f"yt{qt}", tag="yt")
                nc.vector.tensor_copy(yt, tp)
                for n2 in range(D_MODEL // 512):
                    ops = psum_av.tile([128, 512], F32, name=f"op{qt}_{n2}", tag="av")
                    nc.tensor.matmul(
                        ops,
                        lhsT=yt,
                        rhs=wo_sb[:, n2 * 512:(n2 + 1) * 512],
                        start=True,
                        stop=True,
                    )
                    ob = ob_pool.tile([128, 512], F32, name=f"ob{qt}_{n2}", tag="ob")
                    nc.vector.tensor_copy(ob, ops)
                    nc.sync.dma_start(
                        out[qt * 128:(qt + 1) * 128, n2 * 512:(n2 + 1) * 512], ob
                    )

            # --- preamble: projections needed by the first attention step ---
            for g in range(2):
                for half in range(2):
                    emit_qkv_group(g, 0, half)
            emit_v_tile(0)

            # --- attention, software-pipelined; other phases fed in as
            # --- background PE work so no engine sits idle ---
            from collections import deque

            EXP = mybir.ActivationFunctionType.Exp
            bg = deque()

            def queue_v(kt):
                v_pending[kt] = lambda: emit_v_tile(kt)
                bg.append(lambda: ensure_v(kt))

            for kt in range(1, 8):
                queue_v(kt)
            for Q in range(QC):
                if Q + 1 < QC:
                    for g in range(2):
                        for half in range(2):
                            bg.append(
                                lambda g=g, t=Q + 1, hf=half: emit_qkv_group(g, t, hf)
                            )
                    for kt in range(8 * (Q + 1), 8 * (Q + 1) + 8):
                        queue_v(kt)
                if Q > 0:
                    for s in range(8):
                        bg.append(lambda qt=8 * (Q - 1) + s: emit_outproj(qt))
                iters = 2 * (8 * Q + 8)
                bg_total = len(bg)
                bg_emitted = 0
                it_idx = 0

                for h in range(H2):
                    hp = h * 64  # partition base of this head in q_sb/k_sb
                    # tail chunk: stream epilogue+projection out of the loop
                    # as each q-subtile's accumulation finishes, so the kernel
                    # doesn't end with a long serial projection phase
                    streaming = (Q == QC - 1) and (h == H2 - 1)
                    done_s = set()

                    def emit_epilogue(s, h=h, hp=hp, Q=Q):
                        qt = 8 * Q + s
                        avt = av_lo if s < 4 else av_hi
                        col = (s % 4) * 65
                        rec = rec_pool.tile([128, 1], F32, name=f"rec{h}_{qt}", tag="rec")
                        nc.vector.reciprocal(rec, avt[:, col + 64:col + 65])
                        nc.vector.tensor_scalar_mul(
                            y_sb[qt][:, hp:hp + 64], avt[:, col:col + 64], rec
                        )

                    # 8 q-subtiles packed into two 1-bank accumulators:
                    # col layout [65 x 4]: cols 0-63 = PV, col 64 = sum(exp)
                    av_lo = psum_av.tile([128, 260], F32, name=f"avl{h}_{Q}", tag="av")
                    av_hi = psum_av.tile([128, 260], F32, name=f"avh{h}_{Q}", tag="av")
                    pending = []  # AV matmuls delayed one iteration: keeps PE
                    # from stalling on exp(kt) before issuing scores(kt+1)
                    for kt in range(8 * Q + 8):
                        i = kt - 8 * Q  # >= 0 on diagonal tiles
                        off = 128 * i if i > 0 else 0
                        sps = psum_mm.tile([128, 1024], F32, name=f"s{h}_{Q}_{kt}", tag="mm1k")
                        bounds = [(off, 512), (512, 1024)] if off < 512 else [(off, 1024)]
                        for (a, b) in bounds:
                            nc.tensor.matmul(
                                sps[:, a:b],
                                lhsT=k_sb[hp:hp + 64, kt * 128:(kt + 1) * 128],
                                rhs=q_sb[hp:hp + 64, Q * 1024 + a: Q * 1024 + b],
                                start=True,
                                stop=True,
                            )
                        pt = pt_pool.tile([128, 1024], BF16, name=f"pt{h}_{Q}_{kt}", tag="pt")
                        nc.scalar.activation(pt[:, off:1024], sps[:, off:1024], EXP)
                        if i >= 0:
                            nc.vector.tensor_mul(
                                pt[:, off:off + 128], pt[:, off:off + 128], trimask
                            )
                        ensure_v(kt)  # force the deferred V tile if the
                        # background spread hasn't reached it yet
                        for fn in pending:
                            fn()
                        pending = []
                        if streaming and kt - 1 - 8 * Q == 3:
                            # av_lo's accumulation group closed at kt-1
                            # (stop on s=3) -> its 4 subtiles are readable
                            for s in range(4):
                                done_s.add(s)
                                emit_epilogue(s)
                                emit_outproj(8 * Q + s)
                        for s in range(max(i, 0), 8):
                            # start/stop are per 2KB psum zero-region (= one
                            # bank): first/last matmul touching each packed
                            # accumulator, not each 65-col slice
                            pending.append(
                                lambda pt=pt, s=s, kt=kt, avt=(av_lo if s < 4 else av_hi): nc.tensor.matmul(
                                    avt[:, (s % 4) * 65:(s % 4) * 65 + 65],
                                    lhsT=pt[:, s * 128:s * 128 + 128],
                                    rhs=v_sb[h][kt],
                                    start=(kt == 0 and s % 4 == 0),
                                    stop=(kt == 8 * Q + s and s % 4 == 3),
                                )
                            )
                        it_idx += 1
                        while bg and bg_emitted < (it_idx * bg_total + iters - 1) // iters:
                            bg.popleft()()
                            bg_emitted += 1
                    for fn in pending:
                        fn()
                    for s in range(8):
                        if s in done_s:
                            continue
                        emit_epilogue(s)
                        if streaming:
                            emit_outproj(8 * Q + s)
                while bg:
                    bg.popleft()()

    nc.compile()
    return nc


def make_in_maps(x, w_qkv, w_out, T: int = T_FULL):
    """Shard full inputs into the 8 per-core input dicts."""
    x = np.asarray(x, dtype=np.float32)
    w_qkv = np.asarray(w_qkv, dtype=np.float32)
    w_out = np.asarray(w_out, dtype=np.float32)
    xm = x.reshape(-1, D_MODEL)[:T]  # [T, C]
    xt = np.ascontiguousarray(xm.T).astype(NPBF16)  # [C, T]

    CH = D_MODEL // 128
    Wq = w_qkv[0:D_MODEL] * np.float32(1.0 / np.sqrt(HEAD_DIM))
    Wk = w_qkv[D_MODEL:2 * D_MODEL]
    Wv = w_qkv[2 * D_MODEL:3 * D_MODEL]

    in_maps = []
    for c in range(N_CORES):
        r0 = c * 128
        # [256 rows, C] = [q_h0 | q_h1 | k_h0 | k_h1] stacked along rows
        qk_rows = np.concatenate(
            [Wq[r0:r0 + 128], Wk[r0:r0 + 128]], axis=0
        )  # [256, C]
        # -> [C, 256] -> packed [128, CH*256]
        qk_t = qk_rows.T.reshape(CH, 128, 256).transpose(1, 0, 2).reshape(128, CH * 256)
        v_rows = Wv[r0:r0 + 128]  # [128, C] = [v_h0 | v_h1] along rows
        v_t = v_rows.T.reshape(CH, 128, 128).transpose(1, 0, 2).reshape(128, CH * 128)
        wo_t = np.ascontiguousarray(w_out[:, r0:r0 + 128].T)  # [128, C]
        in_maps.append(
            {
                "xt": xt,
                "wqk": np.ascontiguousarray(qk_t).astype(NPBF16),
                "wv": np.ascontiguousarray(v_t).astype(NPBF16),
                "wo": wo_t.astype(NPBF16),
            }
        )
    return in_maps


_program_cache = {}


def get_program(T: int = T_FULL) -> bass.Bass:
    if T not in _program_cache:
        _program_cache[T] = build_program(T)
    return _program_cache[T]


def run_on_hw(x, w_qkv, w_out, trace: bool = False, T: int = T_FULL):
    from concourse.bass_utils import run_bass_kernel_spmd

    nc = get_program(T)
    in_maps = make_in_maps(x, w_qkv, w_out, T)
    res = run_bass_kernel_spmd(nc, in_maps, core_ids=list(range(N_CORES)), trace=trace)
    acc = np.zeros((T, D_MODEL), np.float32)
    for c in range(N_CORES):
        acc += np.asarray(res.results[c]["out_partial"], dtype=np.float32)
    return acc.reshape(1, T, D_MODEL), res


def kernel(x, w_qkv, w_out):
    out, _ = run_on_hw(x, w_qkv, w_out)
    return out.astype(np.float32)


# revision 31
# speedup vs baseline: 341.6052x; 341.6052x over previous
"""Trainium2 Bass kernel for causal multi-head attention (no KV cache).

Problem: x[1,4096,1024], w_qkv[3072,1024], w_out[1024,1024], 16 heads, hd=64.
Sharding: tensor-parallel over heads — each of the 8 cores owns 2 heads.
Per core: QKV projection for its heads, causal softmax(QK^T/8)V, and the
partial output projection  y_c @ w_out[:, c*128:(c+1)*128].T  -> [4096,1024].
Host sums the 8 fp32 partials.

Device layout choices:
  - x is shipped pre-transposed (xT [1024,4096], bf16): the QKV matmuls
    contract over the model dim, which must live on SBUF partitions.
  - q^T and k^T are produced directly as [64, T] (head dim on partitions) by
    making the projection-weight slice the stationary operand.  1/sqrt(hd) is
    folded into Wq on the host.
  - scores are computed transposed, S^T[k,q], so the exp output is already the
    lhsT the PV matmul needs — no on-device transposes in the inner loop.
  - softmax denominator comes free from a ones-column appended to V.
  - causal masking: k-tiles strictly above the diagonal are skipped, the
    diagonal tiles are free-dim-trimmed and masked with a static [128,128]
    triangular multiply.
"""

import os
import sys

import numpy as np

for _p in ("/opt/trn_rl_repo", "/root/.axon_site/_ro/trn_rl_repo"):
    if os.path.isdir(_p) and _p not in sys.path:
        sys.path.insert(0, _p)

import ml_dtypes

import concourse.bass as bass
import concourse.mybir as mybir
import concourse.tile as tile
from concourse import bacc, masks
from concourse._compat import with_exitstack  # noqa: F401  (parity with repo kernels)

BF16 = mybir.dt.bfloat16
F32 = mybir.dt.float32
NPBF16 = ml_dtypes.bfloat16

D_MODEL = 1024
N_HEADS = 16
HEAD_DIM = 64
N_CORES = 8
HEADS_PER_CORE = N_HEADS // N_CORES  # 2
T_FULL = 4096


def build_program(T: int = T_FULL, loop_reps: int | None = None) -> bass.Bass:
    """One NeuronCore's program; all 8 cores run it on different data.

    loop_reps: wrap the whole kernel in a device-side For loop (identical
    iterations) — benchmarking aid: slope of wall-time over reps gives the
    per-iteration device time without host/tunnel dispatch noise."""
    assert T % 1024 == 0
    TT = T // 128          # 128-row T-tiles
    CH = D_MODEL // 128    # c-chunks of the contraction dim
    QC = T // 1024         # 1024-wide q chunks
    H2 = HEADS_PER_CORE

    nc = bacc.Bacc("TRN2", target_bir_lowering=False, debug=False)

    xt = nc.dram_tensor("xt", [D_MODEL, T], BF16, kind="ExternalInput").ap()
    # packed per c-chunk: 256 cols = [q_h0 | q_h1 | k_h0 | k_h1] (64 each)
    wqk = nc.dram_tensor("wqk", [128, CH * 256], BF16, kind="ExternalInput").ap()
    # packed per c-chunk: 128 cols = [v_h0 | v_h1]
    wv = nc.dram_tensor("wv", [128, CH * 128], BF16, kind="ExternalInput").ap()
    # w_out[:, c*128:(c+1)*128].T
    wo = nc.dram_tensor("wo", [128, D_MODEL], BF16, kind="ExternalInput").ap()
    out = nc.dram_tensor("out_partial", [T, D_MODEL], F32, kind="ExternalOutput").ap()

    with tile.TileContext(nc) as tc:
        import contextlib

        with contextlib.ExitStack() as ctx:
            const_pool = ctx.enter_context(tc.tile_pool(name="const", bufs=1))
            xt_pool = ctx.enter_context(tc.tile_pool(name="xt", bufs=1))
            qk_pool = ctx.enter_context(tc.tile_pool(name="qk", bufs=1))
            v_pool = ctx.enter_context(tc.tile_pool(name="v", bufs=1))
            y_pool = ctx.enter_context(tc.tile_pool(name="y", bufs=1))
            pt_pool = ctx.enter_context(tc.tile_pool(name="ptp", bufs=6))
            yt_pool = ctx.enter_context(tc.tile_pool(name="ytp", bufs=3))
            ob_pool = ctx.enter_context(tc.tile_pool(name="obp", bufs=3))
            rec_pool = ctx.enter_context(tc.tile_pool(name="recp", bufs=8))
            # [128,1024] fp32 = 2 banks x 2 bufs = 4 banks
            psum_mm = ctx.enter_context(tc.tile_pool(name="psmm", bufs=2, space="PSUM"))
            # [128,<=260] fp32 = 1 bank x 4 bufs = 4 banks (shared with V/transpose)
            psum_av = ctx.enter_context(tc.tile_pool(name="psav", bufs=4, space="PSUM"))

            if loop_reps:
                ctx.enter_context(tc.For_i(0, loop_reps, 1))

            # --- constants ---
            identity = const_pool.tile([128, 128], F32, name="identity")
            masks.make_identity(nc, identity)
            # trimask[p, g] = 1.0 where p <= g else 0  (keep k <= q)
            trimask = const_pool.tile([128, 128], BF16, name="trimask")
            masks.make_upper_triangular(nc, trimask, val=1.0, diag=True)

            # --- x^T + weight loads.  The first attention chunk only needs
            # --- x columns [0,1024), so DMA T-slice by T-slice: attention
            # --- starts ~6us in instead of waiting for the full 8MB. ---
            xt_sb = []
            for cc in range(CH):
                t = xt_pool.tile([128, T], BF16, name=f"xt{cc}", tag=f"xt{cc}")
                xt_sb.append(t)
            for cc in range(CH):
                nc.sync.dma_start(
                    xt_sb[cc][:, 0:1024], xt[cc * 128:(cc + 1) * 128, 0:1024]
                )
            wqk_sb = const_pool.tile([128, CH * 256], BF16, name="wqk_sb")
            nc.sync.dma_start(wqk_sb, wqk)
            wv_sb = const_pool.tile([128, CH * 128], BF16, name="wv_sb")
            nc.sync.dma_start(wv_sb, wv)
            for tch in range(1, T // 1024):
                for cc in range(CH):
                    nc.sync.dma_start(
                        xt_sb[cc][:, tch * 1024:(tch + 1) * 1024],
                        xt[cc * 128:(cc + 1) * 128, tch * 1024:(tch + 1) * 1024],
                    )
            wo_sb = const_pool.tile([128, D_MODEL], BF16, name="wo_sb")
            nc.sync.dma_start(wo_sb, wo)

            # --- persistent tiles ---
            q_sb = qk_pool.tile([128, T], BF16, name="q_sb", tag="q_sb")
            k_sb = qk_pool.tile([128, T], BF16, name="k_sb", tag="k_sb")
            v_sb = [[None] * TT for _ in range(H2)]
            y_sb = []
            for qt in range(TT):
                t = y_pool.tile([128, 128], F32, name=f"y{qt}", tag=f"y{qt}")
                y_sb.append(t)

            # --- emission helpers (deferred so phases interleave on PE) ---
            def emit_qkv_group(g, tch, half):
                """q^T (g=0) or k^T (g=1) for cols [1024*tch+512*half, +512).
                Uses the 1-bank psum pool so the scores double-buffer in
                psum_mm is never starved by background projection work."""
                dst = q_sb if g == 0 else k_sb
                c0 = tch * 1024 + half * 512
                ps = psum_av.tile([128, 512], F32, name=f"qkps{g}_{tch}_{half}", tag="av")
                for cc in range(CH):
                    nc.tensor.matmul(
                        ps,
                        lhsT=wqk_sb[:, cc * 256 + g * 128: cc * 256 + g * 128 + 128],
                        rhs=xt_sb[cc][:, c0:c0 + 512],
                        start=(cc == 0),
                        stop=(cc == CH - 1),
                    )
                nc.vector.tensor_copy(dst[:, c0:c0 + 512], ps)

            v_pending = {}  # kt -> deferred emit, forced before first use

            def ensure_v(kt):
                fn = v_pending.pop(kt, None)
                if fn is not None:
                    fn()

            def emit_v_tile(kt):
                """v[k-tile, 64+ones] for both heads."""
                vps = psum_av.tile([128, 128], F32, name=f"vps{kt}", tag="av")
                for cc in range(CH):
                    nc.tensor.matmul(
                        vps,
                        lhsT=xt_sb[cc][:, kt * 128:(kt + 1) * 128],
                        rhs=wv_sb[:, cc * 128:(cc + 1) * 128],
                        start=(cc == 0),
                        stop=(cc == CH - 1),
                    )
                for h in range(H2):
                    vt = v_pool.tile([128, 65], BF16, name=f"v{h}_{kt}", tag=f"v{h}_{kt}")
                    nc.vector.tensor_copy(vt[:, 0:64], vps[:, h * 64:(h + 1) * 64])
                    nc.vector.memset(vt[:, 64:65], 1.0)
                    v_sb[h][kt] = vt

            def emit_outproj(qt):
                """partial[q-tile, :] = y^T via PE transpose, then 2 matmuls."""
                tp = psum_av.tile([128, 128], F32, name=f"tp{qt}", tag="av")
                nc.tensor.transpose(tp, y_sb[qt], identity)
                yt = yt_pool.tile([128, 128], BF16, name=f"yt{qt}", tag="yt")
                nc.vector.tensor_copy(yt, tp)
                for n2 in range(D_MODEL // 512):
                    ops = psum_av.tile([128, 512], F32, name=f"op{qt}_{n2}", tag="av")
                    nc.tensor.matmul(
                        ops,
                        lhsT=yt,
                        rhs=wo_sb[:, n2 * 512:(n2 + 1) * 512],
                        start=True,
                        stop=True,
                    )
                    ob = ob_pool.tile([128, 512], F32, name=f"ob{qt}_{n2}", tag="ob")
                    nc.vector.tensor_copy(ob, ops)
                    nc.sync.dma_start(
                        out[qt * 128:(qt + 1) * 128, n2 * 512:(n2 + 1) * 512], ob
                    )

            # --- preamble: projections needed by the first attention step ---
            for g in range(2):
                for half in range(2):
                    emit_qkv_group(g, 0, half)
            emit_v_tile(0)

            # --- attention, software-pipelined; other phases fed in as
            # --- background PE work so no engine sits idle ---
            from collections import deque

            EXP = mybir.ActivationFunctionType.Exp
            bg = deque()

            def queue_v(kt):
                v_pending[kt] = lambda: emit_v_tile(kt)
                bg.append(lambda: ensure_v(kt))

            for kt in range(1, 8):
                queue_v(kt)
            for Q in range(QC):
                if Q + 1 < QC:
                    for g in range(2):
                        for half in range(2):
                            bg.append(
                                lambda g=g, t=Q + 1, hf=half: emit_qkv_group(g, t, hf)
                            )
                    for kt in range(8 * (Q + 1), 8 * (Q + 1) + 8):
                        queue_v(kt)
                if Q > 0:
                    for s in range(8):
                        bg.append(lambda qt=8 * (Q - 1) + s: emit_outproj(qt))
                iters = 2 * (8 * Q + 8)
                bg_total = len(bg)
                bg_emitted = 0
                it_idx = 0

                for h in range(H2):
                    hp = h * 64  # partition base of this head in q_sb/k_sb
                    # tail chunk: stream epilogue+projection out of the loop
                    # as each q-subtile's accumulation finishes, so the kernel
                    # doesn't end with a long serial projection phase
                    streaming = (Q == QC - 1) and (h == H2 - 1)
                    done_s = set()

                    def emit_epilogue(s, h=h, hp=hp, Q=Q):
                        qt = 8 * Q + s
                        avt = av_lo if s < 4 else av_hi
                        col = (s % 4) * 65
                        rec = rec_pool.tile([128, 1], F32, name=f"rec{h}_{qt}", tag="rec")
                        nc.vector.reciprocal(rec, avt[:, col + 64:col + 65])
                        nc.vector.tensor_scalar_mul(
                            y_sb[qt][:, hp:hp + 64], avt[:, col:col + 64], rec
                        )

                    # 8 q-subtiles packed into two 1-bank accumulators:
                    # col layout [65 x 4]: cols 0-63 = PV, col 64 = sum(exp)
                    av_lo = psum_av.tile([128, 260], F32, name=f"avl{h}_{Q}", tag="av")
                    av_hi = psum_av.tile([128, 260], F32, name=f"avh{h}_{Q}", tag="av")
                    pending = []  # AV matmuls delayed one iteration: keeps PE
                    # from stalling on exp(kt) before issuing scores(kt+1)
                    for kt in range(8 * Q + 8):
                        i = kt - 8 * Q  # >= 0 on diagonal tiles
                        off = 128 * i if i > 0 else 0
                        sps = psum_mm.tile([128, 1024], F32, name=f"s{h}_{Q}_{kt}", tag="mm1k")
                        bounds = [(off, 512), (512, 1024)] if off < 512 else [(off, 1024)]
                        for (a, b) in bounds:
                            nc.tensor.matmul(
                                sps[:, a:b],
                                lhsT=k_sb[hp:hp + 64, kt * 128:(kt + 1) * 128],
                                rhs=q_sb[hp:hp + 64, Q * 1024 + a: Q * 1024 + b],
                                start=True,
                                stop=True,
                            )
                        pt = pt_pool.tile([128, 1024], BF16, name=f"pt{h}_{Q}_{kt}", tag="pt")
                        nc.scalar.activation(pt[:, off:1024], sps[:, off:1024], EXP)
                        if i >= 0:
                            nc.vector.tensor_mul(
                                pt[:, off:off + 128], pt[:, off:off + 128], trimask
                            )
                        ensure_v(kt)  # force the deferred V tile if the
                        # background spread hasn't reached it yet
                        for fn in pending:
                            fn()
                        pending = []
                        if streaming and kt - 1 - 8 * Q == 3:
                            # av_lo's accumulation group closed at kt-1
                            # (stop on s=3) -> its 4 subtiles are readable
                            for s in range(4):
                                done_s.add(s)
                                emit_epilogue(s)
                                emit_outproj(8 * Q + s)
                        for s in range(max(i, 0), 8):
                            # start/stop are per 2KB psum zero-region (= one
                            # bank): first/last matmul touching each packed
                            # accumulator, not each 65-col slice
                            pending.append(
                                lambda pt=pt, s=s, kt=kt, avt=(av_lo if s < 4 else av_hi): nc.tensor.matmul(
                                    avt[:, (s % 4) * 65:(s % 4) * 65 + 65],
                                    lhsT=pt[:, s * 128:s * 128 + 128],
                                    rhs=v_sb[h][kt],
                                    start=(kt == 0 and s % 4 == 0),
                                    stop=(kt == 8 * Q + s and s % 4 == 3),
                                )
                            )
                        it_idx += 1
                        while bg and bg_emitted < (it_idx * bg_total + iters - 1) // iters:
                            bg.popleft()()
                            bg_emitted += 1
                    for fn in pending:
                        fn()
                    for s in range(8):
                        if s in done_s:
                            continue
                        emit_epilogue(s)
                        if streaming:
                            emit_outproj(8 * Q + s)
                while bg:
                    bg.popleft()()

    nc.compile()
    return nc


def make_in_maps(x, w_qkv, w_out, T: int = T_FULL):
    """Shard full inputs into the 8 per-core input dicts."""
    x = np.asarray(x, dtype=np.float32)
    w_qkv = np.asarray(w_qkv, dtype=np.float32)
    w_out = np.asarray(w_out, dtype=np.float32)
    xm = x.reshape(-1, D_MODEL)[:T]  # [T, C]
    xt = np.ascontiguousarray(xm.T).astype(NPBF16)  # [C, T]

    CH = D_MODEL // 128
    Wq = w_qkv[0:D_MODEL] * np.float32(1.0 / np.sqrt(HEAD_DIM))
    Wk = w_qkv[D_MODEL:2 * D_MODEL]
    Wv = w_qkv[2 * D_MODEL:3 * D_MODEL]

    in_maps = []
    for c in range(N_CORES):
        r0 = c * 128
        # [256 rows, C] = [q_h0 | q_h1 | k_h0 | k_h1] stacked along rows
        qk_rows = np.concatenate(
            [Wq[r0:r0 + 128], Wk[r0:r0 + 128]], axis=0
        )  # [256, C]
        # -> [C, 256] -> packed [128, CH*256]
        qk_t = qk_rows.T.reshape(CH, 128, 256).transpose(1, 0, 2).reshape(128, CH * 256)
        v_rows = Wv[r0:r0 + 128]  # [128, C] = [v_h0 | v_h1] along rows
        v_t = v_rows.T.reshape(CH, 128, 128).transpose(1, 0, 2).reshape(128, CH * 128)
        wo_t = np.ascontiguousarray(w_out[:, r0:r0 + 128].T)  # [128, C]
        in_maps.append(
            {
                "xt": xt,
                "wqk": np.ascontiguousarray(qk_t).astype(NPBF16),
                "wv": np.ascontiguousarray(v_t).astype(NPBF16),
                "wo": wo_t.astype(NPBF16),
            }
        )
    return in_maps


_program_cache = {}


def get_program(T: int = T_FULL, loop_reps: int | None = None) -> bass.Bass:
    key = (T, loop_reps)
    if key not in _program_cache:
        _program_cache[key] = build_program(T, loop_reps)
    return _program_cache[key]


def run_on_hw(x, w_qkv, w_out, trace: bool = False, T: int = T_FULL):
    from concourse.bass_utils import run_bass_kernel_spmd

    nc = get_program(T)
    in_maps = make_in_maps(x, w_qkv, w_out, T)
    res = run_bass_kernel_spmd(nc, in_maps, core_ids=list(range(N_CORES)), trace=trace)
    acc = np.zeros((T, D_MODEL), np.float32)
    for c in range(N_CORES):
        acc += np.asarray(res.results[c]["out_partial"], dtype=np.float32)
    return acc.reshape(1, T, D_MODEL), res


def kernel(x, w_qkv, w_out):
    out, _ = run_on_hw(x, w_qkv, w_out)
    return out.astype(np.float32)
